# revision 90
# baseline (speedup 1.0000x reference)
"""Multi-head attention (B=2, S=2048, D=1024, H=16 heads, causal) on 8 trn2 cores.

Sharding: 2D — 4 head-groups x 2 batches. Core c = (hg, b) with hg = c//2,
b = c%2 owns heads 4hg..4hg+3 (256 channels) of batch b only:
  - reads only X[b] for q/k/v (12MB vs 25MB for pure head sharding),
  - attention for its 4 heads fully local,
  - W_o row-sharded: partial output [1024, 2048]; host sums the 4 head-group
    partials per batch and adds b_o.

Device layout: channels on partitions, tokens on free dim.
  - Scores as S^T blocks [128 k-tok, <=512 q-tok]; exp is elementwise and the
    softmax denominator rides a ones-column appended to V.
  - Heads processed in pairs (= the two 128-channel chunks). The scores
    stationary is the pair's K block [128ch, 128k]; per-head results come from
    zero-padded Q copies, so one LDWEIGHTS serves both heads.
  - Causal hull: host inspects the mask and streams only [lo, hi) of each
    block column (diagonal blocks shrink), cutting score/AV rows and exp work;
    partial columns are multiplied by a deduplicated 0/1 pattern tile.
  - Projections/output-projection matmuls are woven between attention blocks
    as filler so the PE array never idles (it only reaches max clock after
    ~3us of continuous execution).

All matmuls bf16 (host-cast) with fp32 PSUM accumulation; partial outputs
return bf16 and are reduced in fp32 on the host.
"""

import sys
from collections import deque

import numpy as np

try:
    import concourse.bass as bass  # noqa: F401
except ImportError:  # pragma: no cover
    sys.path.insert(0, "/opt/trn_rl_repo")

import ml_dtypes

import concourse.mybir as mybir
import concourse.tile as tile
from concourse import bacc, bass_utils
from concourse.masks import make_identity

P = 128
B, S, D = 2, 2048, 1024
H, DK = 16, 64
N_CORES = 8
NHG = 4  # head groups
HPC = H // NHG  # heads per core = 4
CH = HPC * DK  # channels per core = 256
NPAIR = 2  # head pairs (= 128-channel chunks) per core
NKB = S // P  # k-blocks = 16
CW = 512  # q column width
NCOL = S // CW  # q columns = 4
NTG = S // CW  # 512-token projection groups = 4
KPG = CW // P  # k-blocks per token group = 4
XC = D // P  # x-dim contraction chunks = 8
MO = D // P  # output-channel chunks = 8

BF16 = mybir.dt.bfloat16
F32 = mybir.dt.float32
NPBF16 = ml_dtypes.bfloat16

_BUILD_CACHE = {}


def _analyze_mask(mask):
    """Block plan from the (1,1,S,S) boolean mask (shared across batch/head).

    plan[j] = tuple of (bk, rlo, hi, mixed) for each k-block with any valid
    entry in q-column j. The matmuls/exp for that block stream q range
    [rlo, hi); mixed = (pat_off, a, w) marks a q-range needing a 0/1
    multiply. The first block of a column is widened to the column hull so
    its start=True matmul zeroes the whole accumulation region. Patterns are
    deduplicated and concatenated into (P, W_total) in [k, q] layout.
    """
    m = np.asarray(mask).reshape(S, S).astype(bool)  # m[q, k]
    pat_index = {}
    pat_list = []
    plan = []
    for j in range(NCOL):
        q0 = j * CW
        raw = []
        for bk in range(NKB):
            sub = m[q0 : q0 + CW, bk * P : (bk + 1) * P]  # (CW q, P k)
            anyv = sub.any(axis=1)
            if not anyv.any():
                continue
            lo = int(np.argmax(anyv))
            hi = CW - int(np.argmax(anyv[::-1]))
            raw.append([bk, lo, hi, sub])
        if raw:
            raw[0][1] = min(r[1] for r in raw)
            raw[0][2] = max(r[2] for r in raw)
        blocks = []
        for bk, lo, hi, sub in raw:
            allv = sub.all(axis=1)
            mixed = None
            notall = ~allv[lo:hi]
            if notall.any():
                a = lo + int(np.argmax(notall))
                b_ = hi - int(np.argmax(notall[::-1]))
                patt = np.ascontiguousarray(sub[a:b_, :].T).astype(np.float32)
                key = (patt.shape[1], patt.tobytes())
                if key not in pat_index:
                    pat_index[key] = len(pat_list)
                    pat_list.append(patt)
                mixed = (pat_index[key], a, b_ - a)
            blocks.append((bk, lo, hi, mixed))
        plan.append(tuple(blocks))
    offs = [0]
    for p_ in pat_list:
        offs.append(offs[-1] + p_.shape[1])
    plan2 = []
    for col in plan:
        col2 = []
        for bk, lo, hi, mixed in col:
            if mixed is not None:
                pid, a, w = mixed
                mixed = (offs[pid], a, w)
            col2.append((bk, lo, hi, mixed))
        plan2.append(tuple(col2))
    if pat_list:
        pat_arr = np.concatenate(pat_list, axis=1)  # (P, W_total)
    else:
        pat_arr = np.ones((P, 1), np.float32)
    return tuple(plan2), pat_arr


def _build(plan, pat_w):
    nc = bacc.Bacc(
        "TRN2",
        target_bir_lowering=False,
        debug=False,
        enable_asserts=True,
        num_devices=N_CORES,
    )
    xq = nc.dram_tensor("xq", [NTG, P, XC, CW], BF16, kind="ExternalInput").ap()
    xk = nc.dram_tensor("xk", [NTG, P, XC, CW], BF16, kind="ExternalInput").ap()
    xv = nc.dram_tensor("xv", [NTG, P, XC, CW], BF16, kind="ExternalInput").ap()
    wq = nc.dram_tensor("wq", [D, CH], BF16, kind="ExternalInput").ap()
    wk = nc.dram_tensor("wk", [D, CH], BF16, kind="ExternalInput").ap()
    wv = nc.dram_tensor("wv", [D, CH], BF16, kind="ExternalInput").ap()
    wo = nc.dram_tensor("wo", [CH, D], BF16, kind="ExternalInput").ap()
    bq = nc.dram_tensor("bq", [CH, 1], F32, kind="ExternalInput").ap()
    bk_ = nc.dram_tensor("bk", [CH, 1], F32, kind="ExternalInput").ap()
    bv = nc.dram_tensor("bv", [CH, 1], F32, kind="ExternalInput").ap()
    mpat = nc.dram_tensor("mpat", [P, pat_w], BF16, kind="ExternalInput").ap()
    out = nc.dram_tensor("out", [MO, NCOL, P, CW], BF16, kind="ExternalOutput").ap()

    with tile.TileContext(nc) as tc:
        with (
            tc.tile_pool(name="const", bufs=1) as const,
            tc.tile_pool(name="persist", bufs=1) as persist,
            tc.tile_pool(name="xt", bufs=4) as xtp,
            tc.tile_pool(name="a2", bufs=6) as a2p,
            tc.tile_pool(name="vst", bufs=2) as vstp,
            tc.tile_pool(name="small", bufs=3) as small,
            tc.tile_pool(name="ps", bufs=1, space="PSUM") as psp,
        ):
            ident = const.tile([P, P], BF16, tag="ident")
            make_identity(nc, ident)
            ones_row = const.tile([1, DK], BF16, tag="ones_row")
            nc.gpsimd.memset(ones_row[:], 1.0)

            w_sb = {}
            b_sb = {}
            # per-projection weight+bias loads, emitted right before the x
            # tensor they gate so the sync queue streams in dependency order
            wdrams = {"k": (wk, bk_), "q": (wq, bq), "v": (wv, bv)}

            def load_w(name, eng):
                wdram, bdram = wdrams[name]
                w_sb[name] = const.tile(
                    [P, XC, CH], BF16, tag=f"w{name}", name=f"w{name}"
                )
                eng.dma_start(
                    w_sb[name][:], wdram.rearrange("(o p) c -> p o c", p=P)
                )
                b_sb[name] = const.tile(
                    [P, NPAIR], F32, tag=f"b{name}", name=f"b{name}"
                )
                eng.dma_start(
                    b_sb[name][:], bdram.rearrange("(c p) o -> p (c o)", p=P)
                )

            mask_sb = const.tile([P, pat_w], BF16, tag="mpat")
            wo_sb = const.tile([P, NPAIR, D], BF16, tag="wo")

            # V with a trailing ones column, per local head: [k, nkb, d+1]
            vaug = []
            for h in range(HPC):
                t = persist.tile(
                    [P, NKB, DK + 1], BF16, tag=f"vaug{h}", name=f"vaug{h}"
                )
                nc.gpsimd.memset(t[:, :, DK : DK + 1], 1.0)
                vaug.append(t)

            # K per pair [128ch, S]; zero-padded Q per (pair, head)
            kt = []
            qz = []
            for pr in range(NPAIR):
                kt.append(
                    persist.tile([P, S], BF16, tag=f"kt{pr}", name=f"kt{pr}")
                )
                qz.append([])
                for hl in range(2):
                    t = persist.tile(
                        [P, S], BF16, tag=f"qz{pr}{hl}", name=f"qz{pr}{hl}"
                    )
                    if hl == 0:
                        nc.gpsimd.memset(t[DK:, :], 0.0)
                    else:
                        nc.gpsimd.memset(t[0:DK, :], 0.0)
                    qz[pr].append(t)
            # qz[pr][hl]: partitions are pair pr's 128 channels; head hl
            # occupies partitions [hl*64:(hl+1)*64], the other 64 stay zero
            # so the pair-packed K stationary contracts to head hl only.

            # normalized attention output per pair [128ch, S] bf16
            yp = [
                persist.tile([P, S], BF16, tag=f"y{pr}", name=f"y{pr}")
                for pr in range(NPAIR)
            ]

            xdr = {"q": xq, "k": xk, "v": xv}
            xt_sb = {}

            def load_x(name, tg, eng=None):
                eng = eng or nc.sync
                t = xtp.tile([P, XC, CW], BF16, tag="xt", name=f"x{name}{tg}")
                for h in range(0, XC, 4):
                    eng.dma_start(t[:, h : h + 4, :], xdr[name][tg, :, h : h + 4, :])
                xt_sb[name, tg] = t

            def proj_chunk_mms(name, tg, cc, ps):
                """The 8 accumulating matmuls of one (proj, tgroup, chunk)."""
                xt = xt_sb[name, tg]
                for xc in range(XC):
                    nc.tensor.matmul(
                        ps[:],
                        lhsT=w_sb[name][:, xc, cc * P : (cc + 1) * P],
                        rhs=xt[:, xc, :],
                        start=(xc == 0),
                        stop=(xc == XC - 1),
                    )

            def proj_kq(name, tg, cc, veng=None):
                veng = nc.vector  # gpsimd cannot read PSUM
                ps = psp.tile([P, CW], F32, tag="pp", bufs=2, name="ps")
                proj_chunk_mms(name, tg, cc, ps)
                tsl = slice(tg * CW, (tg + 1) * CW)
                if name == "k":
                    veng.tensor_add(
                        kt[cc][:, tsl],
                        ps[:],
                        b_sb["k"][:, cc : cc + 1].to_broadcast((P, CW)),
                    )
                else:
                    for hl in range(2):
                        psl = slice(hl * DK, (hl + 1) * DK)
                        veng.tensor_add(
                            qz[cc][hl][psl, tsl],
                            ps[psl, :],
                            b_sb["q"][psl, cc : cc + 1].to_broadcast((DK, CW)),
                        )

            def proj_v(tg, cc, veng=None):
                veng = nc.vector
                ps = psp.tile([P, CW], F32, tag="pp", bufs=2, name="ps")
                proj_chunk_mms("v", tg, cc, ps)
                vst = vstp.tile([P, CW], BF16, tag="vst", name="vst")
                veng.tensor_add(
                    vst[:], ps[:], b_sb["v"][:, cc : cc + 1].to_broadcast((P, CW))
                )
                tp = psp.tile([P, KPG, P], BF16, tag="pp", bufs=2, name="tp")
                for i in range(KPG):
                    nc.tensor.transpose(
                        tp[:, i, :], vst[:, i * P : (i + 1) * P], ident[:]
                    )
                ksl = slice(tg * KPG, (tg + 1) * KPG)
                for hl in range(2):
                    nc.vector.tensor_copy(
                        vaug[cc * 2 + hl][:, ksl, 0:DK],
                        tp[:, :, hl * DK : (hl + 1) * DK],
                    )

            def oproj_unit(mo, col, tail=False):
                ps = psp.tile([P, CW], F32, tag="pp", bufs=2, name="ps")
                csl = slice(col * CW, (col + 1) * CW)
                for cc in range(NPAIR):
                    nc.tensor.matmul(
                        ps[:],
                        lhsT=wo_sb[:, cc, mo * P : (mo + 1) * P],
                        rhs=yp[cc][:, csl],
                        start=(cc == 0),
                        stop=(cc == NPAIR - 1),
                    )
                ob = small.tile([P, CW], BF16, tag="ob", name="ob")
                if tail and mo % 2 == 1:
                    nc.scalar.copy(ob[:], ps[:])
                else:
                    nc.vector.tensor_copy(ob[:], ps[:])
                nc.sync.dma_start(out[mo, col], ob[:])

            deferred = deque()  # pending scale+multiply closures, FIFO

            def attn_col(j, filler, first_pop=True):
                """Attention for q-column j, weaving filler closures between
                blocks. Processes the two head pairs sequentially."""
                blocks = plan[j]
                q0 = j * CW
                for pr in range(NPAIR):
                    if not blocks:
                        nc.gpsimd.memset(yp[pr][:, q0 : q0 + CW], 0.0)
                        continue
                    ops_t = psp.tile(
                        [DK + 1, 2, CW], F32, tag="ops", bufs=1, name=f"ops{j}{pr}"
                    )
                    nblk = len(blocks)
                    pend = deque()

                    def emit_av(i, rlo, hi, bk, a2):
                        for hl in range(2):
                            nc.tensor.matmul(
                                ops_t[:, hl, rlo:hi],
                                lhsT=vaug[pr * 2 + hl][:, bk, :],
                                rhs=a2[:, hl, rlo:hi],
                                start=(i == 0),
                                stop=(i == nblk - 1),
                                skip_group_check=True,
                            )

                    for i, (bk, rlo, hi, mixed) in enumerate(blocks):
                        k0 = bk * P
                        s2 = psp.tile([P, 2, CW], F32, tag="s2", bufs=2, name="s2")
                        for hl in range(2):
                            nc.tensor.matmul(
                                s2[:, hl, rlo:hi],
                                lhsT=kt[pr][:, k0 : k0 + P],
                                rhs=qz[pr][hl][:, q0 + rlo : q0 + hi],
                                start=True,
                                stop=True,
                            )
                        a2 = a2p.tile([P, 2, CW], BF16, tag="a2", name="a2")
                        nc.scalar.activation(
                            a2[:, :, rlo:hi],
                            s2[:, :, rlo:hi],
                            mybir.ActivationFunctionType.Exp,
                            scale=0.125,
                        )
                        if mixed is not None:
                            # 0/1 mask multiply on idle gpsimd (SBUF only);
                            # the AV lag gives it ~2 blocks of slack
                            off, a_, w_ = mixed
                            nc.gpsimd.tensor_tensor(
                                a2[:, :, a_ : a_ + w_],
                                a2[:, :, a_ : a_ + w_],
                                mask_sb[:, None, off : off + w_].to_broadcast(
                                    (P, 2, w_)
                                ),
                                mybir.AluOpType.mult,
                            )
                        pend.append((i, rlo, hi, bk, a2))
                        while len(pend) > 3:
                            emit_av(*pend.popleft())
                        # a deferred scale lands here, once its recip chain
                        # has certainly drained
                        if deferred and i == 2:
                            deferred.popleft()()
                        # first pop immediately, then hold off until block 3
                        # so filler never lands on a PSUM buffer still owned
                        # by the previous column's normalize
                        if filler and ((i == 0 and first_pop) or i >= 3):
                            filler.popleft()()
                    while pend:
                        emit_av(*pend.popleft())
                    # drain the ops bank fast: sums + unnormalized AV to SBUF
                    # (scalar and vector in parallel), so the next pair's AV
                    # accumulation can claim the bank without waiting for the
                    # full normalize chain
                    sums1 = small.tile([1, 2, CW], F32, tag="sums1", name="sums1")
                    nc.vector.tensor_copy(sums1[:], ops_t[DK : DK + 1, :, :])
                    y_un = small.tile([P, CW], BF16, tag="yun", name="yun")
                    nc.scalar.copy(y_un[0:DK, :], ops_t[0:DK, 0, :])
                    nc.vector.tensor_copy(y_un[DK:, :], ops_t[0:DK, 1, :])

                    # reciprocal -> bf16 -> partition broadcast via ones
                    # matmul -> final scale
                    rec1 = small.tile([1, 2, CW], F32, tag="rec1", name="rec1")
                    nc.vector.reciprocal_approx_fast(out=rec1[:], in_=sums1[:])
                    rec1b = small.tile([1, 2, CW], BF16, tag="rec1b", name="rec1b")
                    nc.vector.tensor_copy(rec1b[:], rec1[:])

                    def do_scale(pr=pr, rec1b=rec1b, y_un=y_un):
                        scale_ps = psp.tile(
                            [P, CW], F32, tag="pp", bufs=2, name="scale_ps"
                        )
                        for hl in range(2):
                            nc.tensor.matmul(
                                scale_ps[hl * DK : (hl + 1) * DK, :],
                                lhsT=ones_row[:],
                                rhs=rec1b[0:1, hl, :],
                                start=True,
                                stop=True,
                            )
                        nc.vector.tensor_tensor(
                            yp[pr][:, q0 : q0 + CW],
                            y_un[:],
                            scale_ps[:],
                            mybir.AluOpType.mult,
                        )

                    # defer the scale matmul past the next pair's first
                    # blocks so it never blocks the in-order tensor queue
                    deferred.append(do_scale)

            # ---- schedule ----
            # prologue: k/v stream on the sync queue; q on the scalar queue
            # (idle until the first exp) so projections start back-to-back
            load_w("k", nc.sync)
            load_x("k", 0)
            load_w("q", nc.scalar)
            load_x("q", 0, nc.scalar)
            load_w("v", nc.sync)
            load_x("v", 0)
            nc.scalar.dma_start(mask_sb[:], mpat)
            nc.sync.dma_start(wo_sb[:], wo.rearrange("(c p) m -> p c m", p=P))

            def proj_group(tg):
                fl = deque()
                fl.append(lambda: proj_kq("k", tg, 0))
                fl.append(lambda: proj_kq("k", tg, 1))
                fl.append(lambda: proj_kq("q", tg, 0))
                fl.append(lambda: proj_kq("q", tg, 1))
                fl.append(lambda: proj_v(tg, 0))
                fl.append(lambda: proj_v(tg, 1))
                return fl

            # tg0 projections run straight (nothing to weave into yet)
            for f in proj_group(0):
                f()

            for tg in range(NTG):
                # prefetch next token group's X while col tg attention runs;
                # weave the previous column's output projection and the next
                # group's projections between attention blocks
                filler = deque()
                if tg + 1 < NTG:
                    for name in ("k", "q", "v"):
                        load_x(name, tg + 1)
                    filler.extend(proj_group(tg + 1))
                if tg >= 1:
                    for mo in range(MO):
                        filler.append(
                            lambda mo=mo, col=tg - 1: oproj_unit(mo, col)
                        )
                attn_col(tg, filler, first_pop=(tg + 1 < NTG))
                while filler:
                    filler.popleft()()
            while deferred:
                deferred.popleft()()
            for mo in range(MO):
                oproj_unit(mo, NCOL - 1, tail=True)
    nc.compile()
    return nc


def _get_module(plan, pat_w):
    key = (plan, pat_w)
    if key not in _BUILD_CACHE:
        _BUILD_CACHE[key] = _build(plan, pat_w)
    return _BUILD_CACHE[key]


def _prep_inputs(query, key, value, mask, W_q, b_q, W_k, b_k, W_v, b_v, W_o, b_o):
    def xt_of(x, b):
        x2 = np.asarray(x, np.float32)[b].reshape(S, D)
        xt = x2.T.astype(NPBF16)  # (D, S)
        xt = xt.reshape(XC, P, NTG, CW).transpose(2, 1, 0, 3)
        return np.ascontiguousarray(xt)  # (NTG, P, XC, CW)

    xs = {
        (name, b): xt_of(x, b)
        for name, x in (("q", query), ("k", key), ("v", value))
        for b in range(B)
    }
    plan, pat_arr = _analyze_mask(mask)
    mpat = np.ascontiguousarray(pat_arr).astype(NPBF16)

    W_q = np.asarray(W_q, np.float32)
    W_k = np.asarray(W_k, np.float32)
    W_v = np.asarray(W_v, np.float32)
    W_o = np.asarray(W_o, np.float32)

    in_maps = []
    for c in range(N_CORES):
        hg, b = c // 2, c % 2
        cs = slice(hg * CH, (hg + 1) * CH)
        in_maps.append(
            {
                "xq": xs["q", b],
                "xk": xs["k", b],
                "xv": xs["v", b],
                "wq": np.ascontiguousarray(W_q[cs, :].T).astype(NPBF16),
                "wk": np.ascontiguousarray(W_k[cs, :].T).astype(NPBF16),
                "wv": np.ascontiguousarray(W_v[cs, :].T).astype(NPBF16),
                "wo": np.ascontiguousarray(W_o[:, cs].T).astype(NPBF16),
                "bq": np.asarray(b_q, np.float32)[cs].reshape(CH, 1).copy(),
                "bk": np.asarray(b_k, np.float32)[cs].reshape(CH, 1).copy(),
                "bv": np.asarray(b_v, np.float32)[cs].reshape(CH, 1).copy(),
                "mpat": mpat,
            }
        )
    return plan, mpat.shape[1], in_maps


def run(inputs, trace=False, trace_cores=None):
    """Build (cached), run on 8 cores, return (final_output, BassKernelResults)."""
    plan, pat_w, in_maps = _prep_inputs(**inputs)
    nc = _get_module(plan, pat_w)
    res = bass_utils.run_bass_kernel_spmd(
        nc,
        in_maps,
        core_ids=list(range(N_CORES)),
        trace=trace,
        trace_cores=trace_cores,
    )
    final = np.empty((B, S, D), np.float32)
    b_o = np.asarray(inputs["b_o"], np.float32)
    for b in range(B):
        acc = np.zeros((MO, NCOL, P, CW), np.float32)
        for hg in range(NHG):
            acc += res.results[2 * hg + b]["out"].astype(np.float32)
        acc = acc.transpose(0, 2, 1, 3).reshape(D, S)
        final[b] = acc.T + b_o[None, :]
    return final, res


def kernel(**inputs):
    return run(inputs, trace=False)[0]
